# revision 4
# baseline (speedup 1.0000x reference)
"""NeuralKB retrieval kernel v3 for Trainium2 (Bass/Tile), 8-core SPMD.

Math per score s in {sp,po}, batch b, entity n:
  M[s,b,n] = max_f (2*C[s,n,f] + 2*A[s,b,f] - f2[f] - q2part[s,b])
  out      = min(exp(-0.5*(e2[n] - M)), 1)        (= exp(-0.5*d2min) clamped)

Host precomputes (cheap, O(F*E)): transposed bf16 fact tiles (with ones and
-f2 rows appended), 2*ent^T, e2, and query "moving" matrices whose 30
columns yield W[sec] = 2A - q2 - f2 (16 cols) plus per-score delta columns
w_sec - w_0 / w_sec - w_8 (f2 cancels) for SBUF-only Pool adds.

Device per chunk c (128 facts on partitions, n=512 on free):
  PE:   W group matmuls (30 cols, k<=102) + csp/cpo = fact_chunk^T . 2ent^T
  ACT:  x0 = csp + w0, x8 = cpo + w8 (PSUM->SBUF bf16 casts absorbing two
        sections' adds via bias), + NACT bias-adds from csp PSUM
  Pool: remaining 9 adds via gpsimd tensor_scalar x0/x8 + delta (GPSIMD can
        touch neither PSUM nor 2-tensor ops on real HW)
  DVE:  two merged tensor_tensor max ops (2x bf16): accA (score0), accB
        (score1) over the 16-section xt tile
Chunk 0 writes acc directly (no memset, no max).
Finals in 3 pipelined pieces (accB; accA half 1; half 2): gpsimd
partition_all_reduce(max), per-section DMAs from distinct partitions,
sub/exp/min epilogue split across DVE/ACT/Pool.

Sharding: data-parallel over N (500 entities/core, padded to 512).
"""

import numpy as np

import concourse.bass as bass
import concourse.tile as tile
from concourse import bacc, mybir
from concourse import bass_utils
from concourse.bass_isa import ReduceOp

F32 = mybir.dt.float32
BF16 = mybir.dt.bfloat16
AF = mybir.ActivationFunctionType
ALU = mybir.AluOpType

B = 8
E = 100
F = 4000
FP = 4096
NCHUNK = FP // 128  # 32
NCORE = 512
NSEC = 16
# W columns per chunk: 16 sections + 7 score0 deltas (w_sec - w_0, secs 1..7)
# + 7 score1 deltas (w_sec - w_8, secs 9..15)
NWC = 30
NACT = 5   # ACT-add sections (score0 secs 1..NACT, from csp PSUM)
ALT_SEC = -1  # section that alternates Pool (even chunks) / DVE (odd chunks)
# Pool sections: 16 - 8 - NA2


def build_bass(nact=NACT, repeat=1, groups=4, wcast="a"):
    nc = bacc.Bacc("TRN2", target_bir_lowering=False, debug=False, num_devices=8)

    # f_relT rows: 0..99 fact_rel^T, 100 = ones (x -q2), 101 = -f2 (x ones)
    f_relT = nc.dram_tensor("f_relT", [102, FP], BF16, kind="ExternalInput")
    f_a1T = nc.dram_tensor("f_a1T", [100, FP], BF16, kind="ExternalInput")
    f_a2T = nc.dram_tensor("f_a2T", [100, FP], BF16, kind="ExternalInput")
    ent2T_d = nc.dram_tensor("ent2T", [100, NCORE], BF16, kind="ExternalInput")
    relmov_d = nc.dram_tensor("relmov", [102, NWC], BF16, kind="ExternalInput")
    a1mov_d = nc.dram_tensor("a1mov", [100, NWC], BF16, kind="ExternalInput")
    a2mov_d = nc.dram_tensor("a2mov", [100, NWC], BF16, kind="ExternalInput")
    e2rep_d = nc.dram_tensor("e2rep", [NSEC, NCORE], F32, kind="ExternalInput")
    out = nc.dram_tensor("out", [NSEC, NCORE], F32, kind="ExternalOutput")

    with tile.TileContext(nc) as tc:
        _body(nc, tc, f_relT, f_a1T, f_a2T, ent2T_d, relmov_d, a1mov_d, a2mov_d,
              e2rep_d, out, nact, repeat, groups, wcast)
    nc.compile()
    return nc


def _body(nc, tc, f_relT, f_a1T, f_a2T, ent2T_d, relmov_d, a1mov_d, a2mov_d,
          e2rep_d, out, nact, repeat, groups, wcast):
    import contextlib

    with (
        tc.tile_pool(name="const", bufs=1) as const_pool,
        tc.tile_pool(name="acc", bufs=1) as acc_pool,
        tc.tile_pool(name="wsb", bufs=1) as wsb_pool,
        tc.tile_pool(name="csb", bufs=4) as csb_pool,
        tc.tile_pool(name="xall", bufs=4) as xall_pool,
        tc.tile_pool(name="fin", bufs=1) as fin_pool,
        tc.tile_pool(name="cpsum", bufs=6, space="PSUM") as cpsum_pool,
        tc.tile_pool(name="wpsum", bufs=2, space="PSUM") as wpsum_pool,  # [128, 128] f32 each
    ):
        # small inputs issued from ACT's queue in parallel with SP's fact DMAs
        ent2T = const_pool.tile([100, NCORE], BF16)
        nc.scalar.dma_start(ent2T[:], ent2T_d.ap())
        relmov = const_pool.tile([102, NWC], BF16)
        a1mov = const_pool.tile([100, NWC], BF16)
        a2mov = const_pool.tile([100, NWC], BF16)
        nc.scalar.dma_start(relmov[:], relmov_d.ap())
        nc.scalar.dma_start(a1mov[:], a1mov_d.ap())
        nc.scalar.dma_start(a2mov[:], a2mov_d.ap())

        frelT = const_pool.tile([102, FP], BF16)
        fa1T = const_pool.tile([100, FP], BF16)
        fa2T = const_pool.tile([100, FP], BF16)
        dma_bounds = [0, 2, 8, 16, 24, NCHUNK]
        for g in range(len(dma_bounds) - 1):
            gs = slice(dma_bounds[g] * 128, dma_bounds[g + 1] * 128)
            nc.sync.dma_start(frelT[:, gs], f_relT.ap()[:, gs])
            nc.sync.dma_start(fa1T[:, gs], f_a1T.ap()[:, gs])
            nc.sync.dma_start(fa2T[:, gs], f_a2T.ap()[:, gs])

        nm_e = 8
        nh_e = nm_e // 2
        e2A1 = fin_pool.tile([nh_e, NCORE], F32)
        e2A2 = fin_pool.tile([nm_e - nh_e, NCORE], F32)
        e2B = fin_pool.tile([NSEC - nm_e, NCORE], F32)
        nc.gpsimd.dma_start(e2A1[:], e2rep_d.ap()[0:nh_e, :])
        nc.gpsimd.dma_start(e2A2[:], e2rep_d.ap()[nh_e:nm_e, :])
        nc.gpsimd.dma_start(e2B[:], e2rep_d.ap()[nm_e:NSEC, :])

        accA = acc_pool.tile([128, 8 * NCORE], BF16)  # score0 sections
        accB = acc_pool.tile([128, 8 * NCORE], BF16)  # score1 sections
        W_all = wsb_pool.tile([128, NCHUNK * NWC], F32)

        rep_ctx = tc.For_i(0, repeat, 1) if repeat > 1 else contextlib.nullcontext()
        with rep_ctx:
            _stage1(nc, frelT, fa1T, fa2T, ent2T, relmov, a1mov, a2mov,
                    accA, accB, W_all, cpsum_pool, wpsum_pool, csb_pool,
                    xall_pool, nact, wcast, groups)

        # ---------------- finals (two halves) ----------------
        # Half B = Pool sections [8+na2, 16): acc complete at Pool's last STT,
        # several us before DVE's last merged max -> runs in its shadow.
        # Half A = merge sections [0, 8+na2).
        nm = 8
        nh = nm // 2

        def finish_piece(lo, hi, src, slo, e2t, issuers, tg, min_eng):
            ns = hi - lo
            accmax = acc_pool.tile([128, ns * NCORE], BF16, tag=f"accmax{tg}")
            nc.gpsimd.partition_all_reduce(
                accmax[:], src[:, slo * NCORE : (slo + ns) * NCORE], 128,
                ReduceOp.max,
            )
            m16 = fin_pool.tile([ns, NCORE], BF16, tag=f"m16{tg}")
            # rows of accmax identical; one DMA per section, each from its
            # own source partition and a rotating issuing engine
            for s in range(ns):
                eng = issuers[s % len(issuers)]
                eng.dma_start(
                    m16[s : s + 1, :],
                    accmax[s : s + 1, s * NCORE : (s + 1) * NCORE],
                )
            sub16 = fin_pool.tile([ns, NCORE], F32, tag=f"sub{tg}")
            nc.vector.tensor_tensor(sub16[:], e2t[:], m16[:], op=ALU.subtract)
            exp16 = fin_pool.tile([ns, NCORE], F32, tag=f"exp{tg}")
            nc.scalar.activation(exp16[:], sub16[:], AF.Exp, scale=-0.5)
            out16 = fin_pool.tile([ns, NCORE], F32, tag=f"out{tg}")
            if min_eng == "g":
                nc.gpsimd.tensor_scalar(
                    out=out16[:], in0=exp16[:], scalar1=1.0, scalar2=None,
                    op0=ALU.min,
                )
            else:
                nc.vector.tensor_scalar(
                    out=out16[:], in0=exp16[:], scalar1=1.0, scalar2=None,
                    op0=ALU.min,
                )
            nc.sync.dma_start(out.ap()[lo:hi, :], out16[:])

        finish_piece(nm, NSEC, accB, 0, e2B,
                     (nc.sync, nc.scalar, nc.gpsimd), "B", "g")
        finish_piece(0, nh, accA, 0, e2A1,
                     (nc.sync, nc.scalar, nc.gpsimd), "A1", "v")
        finish_piece(nh, nm, accA, nh, e2A2,
                     (nc.sync, nc.scalar, nc.gpsimd), "A2", "v")


def _stage1(nc, frelT, fa1T, fa2T, ent2T, relmov, a1mov, a2mov,
            accA, accB, W_all, cpsum_pool, wpsum_pool, csb_pool, xall_pool,
            nact, wcast, groups):
    # W-group bounds: fine-grained first group so chunk 0 unblocks early
    wbounds = [0, 2, 8, 16, 24, NCHUNK]

    def emit_w_group(c0, c1):
        # Cols 0..15: W = 2A - q2part - f2 (q2/f2 via the augmented rows of
        # frelT/relmov). Cols 16..22: delta = w_sec - w_8 for the Pool
        # sections (f2 cancels). One PSUM region + one cast per group.
        wg = wpsum_pool.tile([128, (c1 - c0) * NWC], F32, tag="wp")
        for ci, c in enumerate(range(c0, c1)):
            cs = slice(c * 128, (c + 1) * 128)
            ws = wg[:, ci * NWC : (ci + 1) * NWC]
            nc.tensor.matmul(ws, frelT[:, cs], relmov[:], start=True, stop=False)
            nc.tensor.matmul(ws, fa1T[:, cs], a1mov[:], start=False, stop=False)
            nc.tensor.matmul(ws, fa2T[:, cs], a2mov[:], start=False, stop=True)
        nc.scalar.activation(
            W_all[:, c0 * NWC : c1 * NWC], wg[:], AF.Copy
        )

    # emit each W group a few chunks before its first consumer so the PE
    # burst and the ACT cast hide behind steady-state work
    emit_at = {}
    for i in range(len(wbounds) - 1):
        at = 0 if i == 0 else max(wbounds[i - 1] + 1, wbounds[i] - 3)
        emit_at.setdefault(at, []).append(i)
    for c in range(NCHUNK):
        for gi in emit_at.get(c, ()):
            emit_w_group(wbounds[gi], wbounds[gi + 1])
        cs = slice(c * 128, (c + 1) * 128)

        cpo = cpsum_pool.tile([128, NCORE], F32, tag="cp")  # score1 (po): fa1T
        nc.tensor.matmul(cpo[:], fa1T[:, cs], ent2T[:], start=True, stop=True)
        csp = cpsum_pool.tile([128, NCORE], F32, tag="cp")  # score0 (sp): fa2T
        nc.tensor.matmul(csp[:], fa2T[:, cs], ent2T[:], start=True, stop=True)

        first = c == 0
        xt = None
        if not first:
            xt = xall_pool.tile([128, NSEC * NCORE], BF16, tag="xall")

        def secdst(sec):
            if first:
                acc = accA if sec < 8 else accB
                off = sec % 8
                return acc[:, off * NCORE : (off + 1) * NCORE]
            return xt[:, sec * NCORE : (sec + 1) * NCORE]

        def wcol(sec):
            return W_all[:, c * NWC + sec : c * NWC + sec + 1]

        def dcol(sec):  # delta cols: w_sec - w_0 (secs 1..7), w_sec - w_8 (9..15)
            j = NSEC + (sec - 1 if sec < 8 else 7 + sec - 9)
            return W_all[:, c * NWC + j : c * NWC + j + 1]

        # ACT: cast both scores' C to SBUF bf16, absorbing one section's add
        # each via the bias (sections 0 and 8)
        x0 = secdst(0)
        nc.scalar.activation(x0, csp[:], AF.Identity, bias=wcol(0))
        x8 = secdst(8)
        nc.scalar.activation(x8, cpo[:], AF.Identity, bias=wcol(8))
        # Pool adds: score0 secs 1+nact..7 from x0, score1 secs 9..15 from x8
        # (GPSIMD cannot touch PSUM; x0/x8 + delta scalars keep it in SBUF).
        # ALT_SEC moves to DVE on odd chunks for fine balance.
        for sec in list(range(1 + nact, 8)) + list(range(9, NSEC)):
            xbase = x0 if sec < 8 else x8
            if sec == ALT_SEC and c % 2 == 1:
                nc.vector.tensor_scalar(
                    out=secdst(sec), in0=xbase, scalar1=dcol(sec), scalar2=None,
                    op0=ALU.add,
                )
            else:
                nc.gpsimd.tensor_scalar(
                    out=secdst(sec), in0=xbase, scalar1=dcol(sec), scalar2=None,
                    op0=ALU.add,
                )
        # ACT adds: score0 secs 1..nact straight from csp PSUM
        for sec in range(1, 1 + nact):
            nc.scalar.activation(secdst(sec), csp[:], AF.Identity, bias=wcol(sec))
        # DVE: two merged max ops (accB first: its partition-reduce leads)
        if not first:
            nc.vector.tensor_tensor(
                accB[:], accB[:], xt[:, 8 * NCORE :], op=ALU.max
            )
            nc.vector.tensor_tensor(
                accA[:], accA[:], xt[:, : 8 * NCORE], op=ALU.max
            )


_NC_CACHE = None


def get_nc():
    global _NC_CACHE
    if _NC_CACHE is None:
        _NC_CACHE = build_bass()
    return _NC_CACHE


def _bf16(x):
    import ml_dtypes

    return np.asarray(x, dtype=ml_dtypes.bfloat16)


def make_in_maps(rel, arg1, arg2, fact_rel, fact_arg1, fact_arg2, entity_embeddings):
    n_per = F // 8  # 500 real entities per core
    rel = np.asarray(rel, np.float32)
    arg1 = np.asarray(arg1, np.float32)
    arg2 = np.asarray(arg2, np.float32)
    fr = np.asarray(fact_rel, np.float32)
    f1 = np.asarray(fact_arg1, np.float32)
    f2m = np.asarray(fact_arg2, np.float32)
    ent = np.asarray(entity_embeddings, np.float32)

    def pad_fact(m):
        o = np.full((FP, E), 10.0, dtype=np.float32)
        o[:F] = m
        return o

    frp, f1p, f2p = pad_fact(fr), pad_fact(f1), pad_fact(f2m)
    frelT = np.ones((102, FP), np.float32)
    frelT[:100] = frp.T
    f2vec = (frp * frp + f1p * f1p + f2p * f2p).sum(axis=1)  # [FP]
    frelT[101] = -f2vec
    fa1T = f1p.T.copy()
    fa2T = f2p.T.copy()

    q2sp = (rel * rel + arg1 * arg1).sum(axis=1)  # [B]
    q2po = (rel * rel + arg2 * arg2).sum(axis=1)
    relmov = np.ones((102, NWC), np.float32)
    relmov[:100, :B] = 2.0 * rel.T
    relmov[:100, B:NSEC] = 2.0 * rel.T
    relmov[100, :B] = -q2sp
    relmov[100, B:NSEC] = -q2po
    # delta columns (f2 cancels): cols 16..22 = w_sec - w_0 for sp b=1..7,
    # cols 23..29 = w_sec - w_8 for po b=1..7
    relmov[:100, NSEC:] = np.concatenate(
        [2.0 * (rel[1:B] - rel[0]).T, 2.0 * (rel[1:B] - rel[0]).T], axis=1
    )
    relmov[100, NSEC : NSEC + 7] = -(q2sp[1:B] - q2sp[0])
    relmov[100, NSEC + 7 :] = -(q2po[1:B] - q2po[0])
    relmov[101, NSEC:] = 0.0
    a1mov = np.zeros((100, NWC), np.float32)
    a1mov[:, :B] = 2.0 * arg1.T
    a1mov[:, NSEC : NSEC + 7] = 2.0 * (arg1[1:B] - arg1[0]).T
    a2mov = np.zeros((100, NWC), np.float32)
    a2mov[:, B:NSEC] = 2.0 * arg2.T
    a2mov[:, NSEC + 7 :] = 2.0 * (arg2[1:B] - arg2[0]).T

    shared = {
        "f_relT": _bf16(frelT),
        "f_a1T": _bf16(fa1T),
        "f_a2T": _bf16(fa2T),
        "relmov": _bf16(relmov),
        "a1mov": _bf16(a1mov),
        "a2mov": _bf16(a2mov),
    }
    in_maps = []
    for ci in range(8):
        ent_pad = np.zeros((NCORE, E), np.float32)
        ent_pad[:n_per] = ent[ci * n_per : (ci + 1) * n_per]
        e2 = (ent_pad * ent_pad).sum(axis=1)  # [NCORE]
        e2rep = np.broadcast_to(e2, (NSEC, NCORE)).astype(np.float32).copy()
        in_maps.append(
            dict(shared, ent2T=_bf16(2.0 * ent_pad.T), e2rep=e2rep)
        )
    return in_maps


def assemble(results):
    n_per = F // 8
    parts = [r["out"].reshape(2, B, NCORE)[:, :, :n_per] for r in results]
    full = np.concatenate(parts, axis=2)
    return full[0].copy(), full[1].copy()


def kernel(rel, arg1, arg2, fact_rel, fact_arg1, fact_arg2, entity_embeddings):
    nc = get_nc()
    in_maps = make_in_maps(
        rel, arg1, arg2, fact_rel, fact_arg1, fact_arg2, entity_embeddings
    )
    res = bass_utils.run_bass_kernel_spmd(nc, in_maps, core_ids=list(range(8)))
    return assemble(res.results)
